# revision 1
# baseline (speedup 1.0000x reference)
"""Trainium2 Bass kernel for BuildVolume2d (stereo cost volume, L1 over channels).

cost[b, d, h, w] = sum_c |L[b,c,h,w] - R[b,c,h,4w-d]|   (R zero-padded left)

Device identity:  sum_c |L - R| = 2*sum_c max(L, R) - sum_c L - sum_c R.
The DVE computes max(L, R_shifted) for 44 of the 48 (q,t) disparity slices in
ONE tensor_tensor op per h-group (fp16 2x mode, custom overlapping AP); the
PE reduces over channels with 2.0-valued routing stationaries into a dense
(d,h)-row PSUM layout and also subtracts the correction tensor T =
sum_c L + sum_c R_shifted (host-precomputed from the same fp16-quantized
inputs, so the identity is exact) via one -Identity matmul per psum tile.
The remaining 4 slices take a PE/ACT path (psum_d = L - R via +/-Identity
matmuls, ACT Abs(scale=0.5) drain) to offload the DVE, which is the
bottleneck engine at ~97% busy.  ACT drains the final psum to fp16 and its
DGE queue writes the output; input loads stay on the sync DMA queue so
loads never queue behind output stores.

Sharding: batch B=8 -> 8 NeuronCores (one sample per core).  Host prep per
core: fp16 casts, transpose to [(h c), w], phase-split padded R layout
("rall": rall[:, base_t + j] = R[c,h,4j+t], base_0=11, base_t=524t+12,
zero pads), correction tiles t0/t1 in the staged row layout, constants.
For d = 4q + PERM[t] (PERM=[0,3,2,1]) the shifted R column is
rall[(11-q) + 524t + w]; psum row = 32*(d//8 mod 4) + 4*(d%8) + h.
Output is written fp16 and cast to f32 on the host (rel err ~5e-4).
"""
import sys
sys.path.insert(0, '/opt/trn_rl_repo')

import numpy as np
import concourse.bass as bass
import concourse.tile as tile
from concourse import bacc, mybir
from concourse.bass_utils import run_bass_kernel_spmd

# ---- problem constants (hardcoded per spec) ----
B, C, H, W = 8, 32, 256, 512
W4 = 4 * W
D = 48                     # maxdisp
N_CORES = 8
HG = 4                     # h rows per group
N_HG = H // HG             # 64
PW = 524                   # per-phase block width in rall
RALL_W = 2096              # 4 * PW
PERM = [0, 3, 2, 1]        # involution: d = 4q + PERM[t]  <->  t = PERM[d%4]

f16 = mybir.dt.float16
f32 = mybir.dt.float32

PE_QS = 1                  # q'-slices (x4 t) routed to the PE/ACT path

_compiled = None


def build_program(n_hg=N_HG, pe_qs=None):
    if pe_qs is None:
        pe_qs = PE_QS
    nc = bacc.Bacc("TRN2", target_bir_lowering=False, debug=False,
                   num_devices=N_CORES)
    fl = nc.dram_tensor("feat_l", [H * C, W], f16, kind="ExternalInput").ap()
    fr = nc.dram_tensor("rall", [H * C, RALL_W], f16, kind="ExternalInput").ap()
    t0d = nc.dram_tensor("t0", [N_HG, 128, W], f16, kind="ExternalInput").ap()
    t1d = nc.dram_tensor("t1", [N_HG, 64, W], f16, kind="ExternalInput").ap()
    std = nc.dram_tensor("st", [128, 256], f16, kind="ExternalInput").ap()
    nid = nc.dram_tensor("negI", [128, 128], f16, kind="ExternalInput").ap()
    pid = nc.dram_tensor("posI", [128, 128], f16, kind="ExternalInput").ap()
    out = nc.dram_tensor("cost", [D, H, W], f16, kind="ExternalOutput").ap()

    with tile.TileContext(nc) as tc:
        with (
            tc.tile_pool(name="const", bufs=1) as constp,
            tc.tile_pool(name="inp", bufs=3) as inp,
            tc.tile_pool(name="tp", bufs=4) as tp,
            tc.tile_pool(name="difp", bufs=2) as difp,
            tc.tile_pool(name="outp", bufs=4) as outp,
            tc.tile_pool(name="outp1", bufs=4) as outp1,
            tc.tile_pool(name="ps0", bufs=2, space="PSUM") as ps0,
            tc.tile_pool(name="psd", bufs=3, space="PSUM") as psd,
            tc.tile_pool(name="abp", bufs=5) as abp,
            tc.tile_pool(name="ps1", bufs=2, space="PSUM") as ps1,
        ):
            def emit_loads(g):
                l16 = inp.tile([128, W], f16, name="l16", tag="l16")
                nc.sync.dma_start(l16[:], fl[128 * g:128 * (g + 1), :])
                rall = inp.tile([128, RALL_W], f16, name="rall", tag="rall")
                nc.sync.dma_start(rall[:], fr[128 * g:128 * (g + 1), :])
                t0 = tp.tile([128, W], f16, name="t0", tag="t0")
                nc.sync.dma_start(t0[:], t0d[g])
                t1 = tp.tile([64, W], f16, name="t1", tag="t1")
                nc.sync.dma_start(t1[:], t1d[g])
                return l16, rall, t0, t1

            def emit_compute(g, l16, rall, t0, t1):
                """Max pass + matmuls + psum->SBUF drain; returns drain state."""
                nv = 12 - pe_qs
                dif = difp.tile([128, nv, 4, W], f16, name="dif")
                # in1: overlapping AP over rall: index = q' + 524*t + w
                in1 = rall[:].copy()
                in1.ap = mybir.VecI64Pair(
                    [[RALL_W, 128], [1, nv], [PW, 4], [1, W]])
                in0 = l16[:].unsqueeze(1).unsqueeze(1) \
                    .broadcast_to((128, nv, 4, W))
                nc.vector.tensor_tensor(dif[:], in0, in1,
                                        op=mybir.AluOpType.max)

                p0 = ps0.tile([128, W], f32, name="p0")
                p1 = ps1.tile([64, W], f32, name="p1")

                # PE/ACT path for offloaded q' slices: psum_d = L - R_slice
                # (2 identity matmuls), ab = |psum_d| * 0.5 on ACT.  Emitted
                # first so the ACT results are ready by the time the trailing
                # ab reduce-matmuls run; reduce-matmuls for these d's go LAST.
                offl = []      # (d, ab tile)
                main = []      # (d, dif slice)
                for d in range(D):
                    q = d // 4
                    t = PERM[d % 4]
                    qp = 11 - q
                    if qp >= nv:
                        pd = psd.tile([128, W], f32, name="pd")
                        rsl = rall[:].copy()
                        rsl.ap = mybir.VecI64Pair([[RALL_W, 128], [1, W]])
                        rsl.offset = qp + PW * t
                        nc.tensor.matmul(pd[:], pi[:], l16[:],
                                         start=True, stop=False,
                                         skip_group_check=True)
                        nc.tensor.matmul(pd[:], ni[:], rsl,
                                         start=False, stop=True,
                                         skip_group_check=True)
                        ab = abp.tile([128, W], f16, name="ab")
                        nc.scalar.activation(
                            ab[:], pd[:], mybir.ActivationFunctionType.Abs,
                            scale=0.5)
                        offl.append((d, ab[:]))
                    else:
                        main.append((d, dif[:, qp, t, :]))

                # Emit each 32-row block contiguously (psum groups may not
                # overlap within a tile); blocks containing offloaded d's go
                # last so their ACT-produced movers are ready.
                by_blk = {}
                for d, mov in main + offl:
                    by_blk.setdefault(d // 8, []).append((d, mov))
                has_off = {d // 8 for d, _ in offl}
                order = []
                for blk in sorted(by_blk, key=lambda b: (b in has_off, b)):
                    order.extend(by_blk[blk])
                emitted = {}
                last_of_blk = {}
                for d, _ in order:
                    last_of_blk[d // 8] = d
                for d, mov in order:
                    j = d % 8
                    blk = d // 8
                    if blk < 4:
                        dst = p0[32 * blk:32 * (blk + 1), :]
                        tpos = (0, 32 * blk)
                    else:
                        dst = p1[32 * (blk - 4):32 * (blk - 3), :]
                        tpos = (0, 32 * (blk - 4))
                    nc.tensor.matmul(dst, st[:, 32 * j:32 * (j + 1)], mov,
                                     start=not emitted.get(blk, False),
                                     stop=(d == last_of_blk[blk]),
                                     tile_position=tpos)
                    emitted[blk] = True

                # psum -= T via PE (stationary = -Identity, moving = T tile)
                nc.tensor.matmul(p0[:], ni[:], t0[:],
                                 start=False, stop=True, skip_group_check=True)
                nc.tensor.matmul(p1[:], ni[0:64, 0:64], t1[:64, :],
                                 start=False, stop=True, skip_group_check=True)

                h0 = HG * g
                o0 = outp.tile([128, W], f16, name="o0", tag="o0")
                nc.scalar.copy(o0[:], p0[:])
                o1 = outp1.tile([64, W], f16, name="o1", tag="o1")
                nc.scalar.copy(o1[:], p1[:])
                nc.scalar.dma_start(
                    out[0:32, h0:h0 + HG, :]
                    .rearrange("(a b) h w -> a b h w", a=4),
                    o0[:])
                nc.scalar.dma_start(
                    out[32:48, h0:h0 + HG, :]
                    .rearrange("(a b) h w -> a b h w", a=2),
                    o1[:])

            q0 = emit_loads(0)
            st = constp.tile([128, 256], f16, name="st")
            nc.sync.dma_start(st[:], std[:])
            ni = constp.tile([128, 128], f16, name="ni")
            nc.sync.dma_start(ni[:], nid[:])
            pi = constp.tile([128, 128], f16, name="pi")
            nc.sync.dma_start(pi[:], pid[:])
            q1 = emit_loads(1) if n_hg > 1 else None
            for g in range(n_hg):
                nxt = emit_loads(g + 2) if g + 2 < n_hg else None
                emit_compute(g, *q0)
                q0, q1 = q1, nxt
    nc.compile()
    return nc


def make_stationaries():
    # st[:, 32j + m] = 2.0 where m = 4j' ... value 2.0 at (h*32+c, 4*j + h)
    st = np.zeros((128, 256), np.float16)
    for j in range(8):
        for h in range(4):
            st[h * 32:(h + 1) * 32, 32 * j + 4 * j + h] = 2.0
    return st


def host_prep(feat_l, feat_r):
    """Per-core input maps: fp16 L, phase-split padded fp16 rall, correction
    tiles t0/t1 in the exact staged row layout, and the stationaries."""
    st = make_stationaries()
    negI = (-np.eye(128)).astype(np.float16)
    posI = np.eye(128).astype(np.float16)
    maps = []
    for i in range(N_CORES):
        l16 = np.ascontiguousarray(
            feat_l[i].transpose(1, 0, 2)).reshape(H * C, W).astype(np.float16)
        r16 = np.ascontiguousarray(
            feat_r[i].transpose(1, 0, 2)).reshape(H * C, W4).astype(np.float16)

        rall = np.zeros((H * C, RALL_W), np.float16)
        for t in range(4):
            base = 11 if t == 0 else PW * t + 12
            rall[:, base:base + W] = r16[:, t::4]

        # correction T[d, h, w] = sum_c L + sum_c R[.., 4w-d] (0 when 4w-d<0),
        # computed from the SAME fp16-quantized values, f32 accumulation.
        l32 = l16.astype(np.float32).reshape(H, C, W)
        r32 = r16.astype(np.float32).reshape(H, C, W4)
        SL = l32.sum(axis=1)                      # [H, W]
        SR = r32.sum(axis=1)                      # [H, W4]
        d_idx = np.arange(D)[:, None]             # [D, 1]
        w_idx = 4 * np.arange(W)[None, :]         # [1, W]
        gidx = w_idx - d_idx                      # [D, W] = 4w - d
        valid = gidx >= 0
        g = np.clip(gidx, 0, W4 - 1)
        SRg = SR[:, g] * valid[None, :, :]        # [H, D, W]
        T = SL[:, None, :] + SRg                  # [H, D, W]
        T = np.ascontiguousarray(T.transpose(1, 0, 2))  # [D, H, W]

        # scatter into staged row layout: row = 32*(d//8 % 4) + 4*(d%8) + h
        Thg = T.reshape(D, N_HG, HG, W)           # [D, 64, 4, W]
        t0 = np.empty((N_HG, 128, W), np.float16)
        t1 = np.empty((N_HG, 64, W), np.float16)
        for d in range(D):
            if (11 - d // 4) >= 12 - PE_QS:
                Thg[d] = 0.0
            j = d % 8
            blk = d // 8
            if blk < 4:
                t0[:, 32 * blk + 4 * j:32 * blk + 4 * j + 4, :] = Thg[d]
            else:
                t1[:, 32 * (blk - 4) + 4 * j:32 * (blk - 4) + 4 * j + 4, :] \
                    = Thg[d]
        maps.append({"feat_l": l16, "rall": rall, "t0": t0, "t1": t1,
                     "st": st, "negI": negI, "posI": posI})
    return maps


def kernel(feat_l, feat_r, maxdisp):
    global _compiled
    feat_l = np.asarray(feat_l, dtype=np.float32)
    feat_r = np.asarray(feat_r, dtype=np.float32)
    assert int(maxdisp) == D
    assert feat_l.shape == (B, C, H, W) and feat_r.shape == (B, C, H, W4)
    if _compiled is None:
        _compiled = build_program()
    in_maps = host_prep(feat_l, feat_r)
    res = run_bass_kernel_spmd(_compiled, in_maps, list(range(N_CORES)))
    return np.stack(
        [res.results[i]["cost"].astype(np.float32) for i in range(N_CORES)],
        axis=0)

